# revision 52
# baseline (speedup 1.0000x reference)
"""Trainium2 Bass kernel for YOLO-style DetectionLoss.

Contract: kernel(**inputs) takes the FULL inputs (batch 512) and returns the
full output (5-tuple of f32 scalars), sharding batch-wise across 8 NeuronCores.

Device-side strategy (per core: 64 images, 2048 GTs):
  - the noobj term needs only channels {0,5} (objectness logits) of every
    cell; they ship as a compact fp8 stream [128, 2704] (346 KB vs the
    12.5 MB full f32 shard), softplus'd on ACT as ln(1+e^x) with the ln
    pass shrunk 4x by a bf16 pairwise product tree on DVE:
    sum ln(1+e^x) = sum ln(prod_4 (1+e^x))
  - the 2048 GT cells are host-gathered into channel-blocked packs (txy,
    twh in f32 -- they decide the responsible-box argmax ties -- cls, obj
    in bf16) merged with the gt meta by dtype, so the device pays three
    DMAs total instead of 16 indirect row-gathers, and the decode exps
    are contiguous ACT instructions
  - per-GT work (sigmoid decode, IoU responsible-box pick, coord/obj/class
    losses, noobj dedup correction) runs on DVE/ACT mirroring the
    reference's math; the IoU compare uses i1*A0 > i0*A1 (the i0*i1 terms
    of the cross-multiplied union compare cancel)
  - gt-derived bookkeeping (cell indices, corner boxes, sqrt targets,
    first-GT-in-cell dedup mask, validity-masked one-hot labels) is
    host-precomputed from the small gt tensors; exact-in-bf16 parts ship
    as bf16
  - accumulators land in one [128, 8] stats tile via accum_out (DVE cols
    0-3,5; ACT col 4), reduced across partitions with a ones-vector
    matmul; the host sums the 8 per-core rows.

Scheduling notes: Tile orders each engine's stream from its own cost
estimates, so two dep edges pin the critical path -- the stream pairing
(q) behind the responsible-box chain on DVE, and the big stream ln (lnm)
behind round-2 on ACT. DMAs split across the sync and scalar HWDGE rings;
the gpsimd SWDGE path measured ~54 GB/s and is avoided entirely.
"""
import sys

sys.path.insert(0, "/opt/trn_rl_repo")

import numpy as np
import ml_dtypes

import concourse.bass as bass
import concourse.tile as tile
from concourse import bacc, mybir
from concourse.tile import add_dep_helper

S = 52
NBOX = 2
NCLS = 8
EPS = 1e-6
LAMBDA_COORD = 5.0
LAMBDA_NOOBJ = 0.5
BATCH = 512
N_GT = 32
NCORES = 8
NIMG = BATCH // NCORES          # 64 images per core
CELLS = S * S                   # 2704
ROWS = NIMG * CELLS             # 173056 cells per core
NG = NIMG * N_GT                # 2048 GTs per core
P = 128
JJ = NG // P                    # 16 GTs per partition
NOBJ = ROWS * NBOX // P         # 2704 obj logits per partition
HALF = NOBJ // 2                # 1352
QRT = NOBJ // 4                 # 676

f32 = mybir.dt.float32
bf16 = mybir.dt.bfloat16
f8 = mybir.dt.float8e4
Act = mybir.ActivationFunctionType
Op = mybir.AluOpType
AxX = mybir.AxisListType.X

# mf (f32, [P, 280]): gathered-cell txy/twh + f32 gt meta, one DMA
C_TXY = 0                       # 64: (j,k,c) j-major
C_TWH = 64                      # 64
M_ONE = 128                     # ones (matmul reduce vector)
M_EPS = 129                     # EPS (activation bias AP)
M_G1 = 136                      # 32: gt corner lo (x,y), j-major
M_G2 = 168                      # 32: gt corner hi (x,y), j-major
M_A2E = 200                     # 16: gt area + EPS
M_SYG = 216                     # 32: (sqrt(w+eps), sqrt(h+eps)), j-major
M_GXY = 248                     # 32: (cx, cy), j-major
MW = 280

# mb (bf16, [P, 352]): gathered-cell cls/obj + bf16-exact gt meta, one DMA
CB_CLS = 0                      # 128: (j,c) j-major
CB_OBJ = 128                    # 32: (j,k)
B_GJS = 160                     # 32: (gj, gi), j-major
B_WV = 192                      # 16: valid & first-GT-in-cell dedup weight
B_GTV = 208                     # 16: gt valid
B_OH = 224                      # 128: validity-masked class one-hot
MBW = 352

_ACT_PATCHED = False


def _force_single_act_table():
    """Place every activation in natural_log_exp_and_others (covers Exp+Ln)
    so the kernel pays one ACT_TABLE_LOAD."""
    global _ACT_PATCHED
    if _ACT_PATCHED:
        return
    from concourse import hw_specs

    orig = hw_specs.get_activation_tables

    def patched(arch):
        t = orig(arch)
        keep = "natural_log_exp_and_others"
        if keep not in t:
            return t
        return {k: (v if k == keep else set()) for k, v in t.items()}

    hw_specs.get_activation_tables = patched
    bacc.get_activation_tables = patched
    _ACT_PATCHED = True


def build_program(for_sim: bool = False) -> bass.Bass:
    _force_single_act_table()
    nc = bacc.Bacc(None, target_bir_lowering=False)

    mf_d = nc.dram_tensor("mf", [P, MW], f32, kind="ExternalInput")
    mb_d = nc.dram_tensor("mb", [P, MBW], bf16, kind="ExternalInput")
    obj_d = nc.dram_tensor("obj", [P, NOBJ], f8, kind="ExternalInput")
    out_d = nc.dram_tensor("out", [1, 8], f32, kind="ExternalOutput")

    with tile.TileContext(nc) as tc:
        with (
            tc.tile_pool(name="main", bufs=1) as mp,
            tc.tile_pool(name="psum", bufs=1, space="PSUM") as pp,
        ):
            stats = mp.tile([P, 8], f32)

            # ---- DMA, 3 transfers total (each dma_start costs ~0.65us of
            #      ring-issue time): the chain-gating f32 pack then the big
            #      fp8 stream on the sync ring; the bf16 pack on the scalar
            #      ring
            meta = mp.tile([P, MW], f32)
            nc.sync.dma_start(out=meta[:], in_=mf_d[:])
            objt = mp.tile([P, NOBJ], f8)
            nc.sync.dma_start(out=objt[:], in_=obj_d[:])
            metab = mp.tile([P, MBW], bf16)
            nc.scalar.dma_start(out=metab[:], in_=mb_d[:])
            cells = meta
            cellsb = metab

            # meta views
            g13 = meta[:, M_G1:M_G1 + 2 * JJ].rearrange("p (j c) -> p j c", c=2)
            g23 = meta[:, M_G2:M_G2 + 2 * JJ].rearrange("p (j c) -> p j c", c=2)
            a2e = meta[:, M_A2E:M_A2E + JJ]
            syg3 = meta[:, M_SYG:M_SYG + 2 * JJ].rearrange(
                "p (j c) -> p j c", c=2)
            gxy3 = meta[:, M_GXY:M_GXY + 2 * JJ].rearrange(
                "p (j c) -> p j c", c=2)
            gjs3 = metab[:, B_GJS:B_GJS + 2 * JJ].rearrange(
                "p (j c) -> p j c", c=2)
            wv = metab[:, B_WV:B_WV + JJ]
            gtv = metab[:, B_GTV:B_GTV + JJ]
            oh3 = metab[:, B_OH:B_OH + NCLS * JJ].rearrange(
                "p (j c) -> p j c", c=NCLS)

            # cells views
            cls3 = cellsb[:, CB_CLS:CB_CLS + 128].rearrange(
                "p (j c) -> p j c", c=NCLS)
            cobj3 = cellsb[:, CB_OBJ:CB_OBJ + 32].rearrange(
                "p (j k) -> p j k", k=2)

            # ---- ACT: decode exps first (they gate the long DVE chain).
            #      txy ships negated in mf, so one Exp over [P,128] yields
            #      e^{-txy} | e^{twh} together.
            exw = mp.tile([P, 128], f32)   # e^{-txy}(64) | e^{twh}(64)
            nc.scalar.activation(
                out=exw[:], in_=cells[:, 0:128], func=Act.Exp)
            exy = exw[:, 0:64]
            ewh = exw[:, 64:128]
            eco = mp.tile([P, 160], f32)   # e^{cls}(128) | e^{obj}(32)
            nc.scalar.activation(out=eco[:], in_=cellsb[:, 0:160], func=Act.Exp)
            scn = mp.tile([P, 32], f32)    # softplus(obj logits) at GT cells
            nc.scalar.activation(
                out=scn[:], in_=eco[:, 128:160], func=Act.Ln, bias=1.0)
            et = mp.tile([P, NOBJ], bf16)
            nc.scalar.activation(out=et[:], in_=objt[:], func=Act.Exp)

            ecls3 = eco[:, 0:128].rearrange("p (j c) -> p j c", c=NCLS)

            # ---- DVE per-GT mid chain: decode, IoU, responsible pick
            den = mp.tile([P, 64], f32)
            nc.vector.tensor_scalar(den[:], exy, 1.0, None, Op.add)
            sgm = mp.tile([P, 64], f32)
            # den = 1+e^{-t} in (1, ~250): no edge cases; ~51 ULP is far
            # below the responsible-box tie noise, ~5x faster than reciprocal
            nc.vector.reciprocal_approx_fast(out=sgm[:], in_=den[:])
            sgm4 = sgm[:].rearrange("p (j k c) -> p j k c", k=2, c=2)
            pb = mp.tile([P, 128], f32)
            pb4 = pb[:].rearrange("p (j k m) -> p j k m", k=2, m=4)
            pbv = pb[:].rearrange("p (t m) -> p t m", m=4)
            # px = (sigmoid + gj) * (1/S), matching the reference's order
            gjb = gjs3.unsqueeze(2).to_broadcast([P, JJ, 2, 2])
            sgp = mp.tile([P, 64], f32)
            sgp4 = sgp[:].rearrange("p (j k c) -> p j k c", k=2, c=2)
            nc.vector.tensor_tensor(sgp4, sgm4, gjb, op=Op.add)
            nc.vector.tensor_scalar(
                pbv[:, :, 0:2], sgp[:].rearrange("p (t c) -> p t c", c=2),
                1.0 / S, None, Op.mult)
            nc.vector.tensor_scalar(
                pbv[:, :, 2:4], ewh.rearrange("p (t c) -> p t c", c=2),
                1.0, None, Op.min)
            p1 = mp.tile([P, 64], f32)
            p14 = p1[:].rearrange("p (j k c) -> p j k c", k=2, c=2)
            nc.vector.scalar_tensor_tensor(
                out=p1[:].rearrange("p (t c) -> p t c", c=2),
                in0=pbv[:, :, 2:4], scalar=-0.5,
                in1=pbv[:, :, 0:2], op0=Op.mult, op1=Op.add)
            p2 = mp.tile([P, 64], f32)
            p24 = p2[:].rearrange("p (j k c) -> p j k c", k=2, c=2)
            nc.vector.scalar_tensor_tensor(
                out=p2[:].rearrange("p (t c) -> p t c", c=2),
                in0=pbv[:, :, 2:4], scalar=0.5,
                in1=pbv[:, :, 0:2], op0=Op.mult, op1=Op.add)
            g1b = g13.unsqueeze(2).to_broadcast([P, JJ, 2, 2])
            g2b = g23.unsqueeze(2).to_broadcast([P, JJ, 2, 2])
            lo = mp.tile([P, 64], f32)
            lo4 = lo[:].rearrange("p (j k c) -> p j k c", k=2, c=2)
            nc.vector.tensor_tensor(lo4, p14, g1b, op=Op.max)
            hi = mp.tile([P, 64], f32)
            hi4 = hi[:].rearrange("p (j k c) -> p j k c", k=2, c=2)
            nc.vector.tensor_tensor(hi4, p24, g2b, op=Op.min)
            iwr = mp.tile([P, 64], f32)
            nc.vector.tensor_tensor(iwr[:], hi[:], lo[:], op=Op.subtract)
            iwh = mp.tile([P, 64], f32)
            nc.vector.tensor_scalar(iwh[:], iwr[:], 0.0, None, Op.max)
            iwh4 = iwh[:].rearrange("p (j k c) -> p j k c", k=2, c=2)
            inter = mp.tile([P, 32], f32)
            inter3 = inter[:].rearrange("p (j k) -> p j k", k=2)
            nc.vector.tensor_tensor(
                inter3, iwh4[:, :, :, 0], iwh4[:, :, :, 1], op=Op.mult)
            a1 = mp.tile([P, 32], f32)
            a13 = a1[:].rearrange("p (j k) -> p j k", k=2)
            nc.vector.tensor_tensor(
                a13, pb4[:, :, :, 2], pb4[:, :, :, 3], op=Op.mult)
            # iou1 > iou0  <=>  i1*(A0-i0) > i0*(A1-i1)  <=>  i1*A0 > i0*A1
            # (A_k = area_k + area_gt + EPS; the i0*i1 terms cancel)
            a2b = a2e.unsqueeze(2).to_broadcast([P, JJ, 2])
            u1 = mp.tile([P, 32], f32)
            u13 = u1[:].rearrange("p (j k) -> p j k", k=2)
            nc.vector.tensor_tensor(u13, a13, a2b, op=Op.add)
            d0 = mp.tile([P, JJ], f32)
            nc.vector.tensor_tensor(
                d0[:], inter3[:, :, 0], u13[:, :, 1], op=Op.mult)
            d1 = mp.tile([P, JJ], f32)
            nc.vector.tensor_tensor(
                d1[:], inter3[:, :, 1], u13[:, :, 0], op=Op.mult)
            sel = mp.tile([P, JJ], f32)
            nc.vector.tensor_tensor(sel[:], d1[:], d0[:], op=Op.is_gt)
            # responsible obj logit first: it unblocks ACT's eo/so pair
            od = mp.tile([P, JJ], f32)
            nc.vector.tensor_tensor(
                od[:], cobj3[:, :, 1], cobj3[:, :, 0], op=Op.subtract)
            om = mp.tile([P, JJ], f32)
            nc.vector.tensor_tensor(om[:], od[:], sel[:], op=Op.mult)
            btob = mp.tile([P, JJ], f32)
            btob_i = nc.vector.tensor_tensor(
                btob[:], om[:], cobj3[:, :, 0], op=Op.add)
            selb4 = sel[:].unsqueeze(2).to_broadcast([P, JJ, 4])
            bd = mp.tile([P, 64], f32)
            bd3 = bd[:].rearrange("p (j m) -> p j m", m=4)
            nc.vector.tensor_tensor(
                bd3, pb4[:, :, 1, :], pb4[:, :, 0, :], op=Op.subtract)
            bm = mp.tile([P, 64], f32)
            bm3 = bm[:].rearrange("p (j m) -> p j m", m=4)
            nc.vector.tensor_tensor(bm3, bd3, selb4, op=Op.mult)
            b = mp.tile([P, 64], f32)
            b3 = b[:].rearrange("p (j m) -> p j m", m=4)
            nc.vector.tensor_tensor(b3, bm3, pb4[:, :, 0, :], op=Op.add)

            # coord xy part into packed d2 tile (j, [dx dy dw dh])
            d2 = mp.tile([P, 64], f32)
            d24 = d2[:].rearrange("p (j m) -> p j m", m=4)
            dxy = mp.tile([P, 32], f32)
            dxy3 = dxy[:].rearrange("p (j c) -> p j c", c=2)
            nc.vector.tensor_tensor(
                dxy3, b3[:, :, 0:2], gxy3, op=Op.subtract)
            nc.vector.tensor_tensor(d24[:, :, 0:2], dxy3, dxy3, op=Op.mult)

            # stream pairing (bf16 2x): q = 1 + e^x, two pairing levels so
            # the ACT ln pass covers NOBJ/4 elements; pinned behind btob so
            # it cannot preempt the latency-critical chain
            q = mp.tile([P, NOBJ], bf16)
            q_i = nc.vector.tensor_scalar(q[:], et[:], 1.0, None, Op.add)
            add_dep_helper(q_i.ins, btob_i.ins, False,
                           "stream pairing after the responsible-box chain")
            m1 = mp.tile([P, HALF], bf16)
            nc.vector.tensor_tensor(
                m1[:], q[:, 0:HALF], q[:, HALF:NOBJ], op=Op.mult)
            m2 = mp.tile([P, QRT], bf16)
            nc.vector.tensor_tensor(
                m2[:], m1[:, 0:QRT], m1[:, QRT:HALF], op=Op.mult)
            # class sums: cls_loss = sum(ls*v) - sum(cls*onehot_masked)
            sm = mp.tile([P, JJ], f32)
            nc.vector.tensor_reduce(sm[:], ecls3, axis=AxX, op=Op.add)
            pickv = mp.tile([P, NCLS * JJ], f32)
            pickv3 = pickv[:].rearrange("p (j c) -> p j c", c=NCLS)
            nc.vector.scalar_tensor_tensor(
                out=pickv3, in0=oh3, scalar=1.0, in1=cls3,
                op0=Op.mult, op1=Op.mult, accum_out=stats[:, 5:6])
            # noobj dedup correction
            wvb = wv.unsqueeze(2).to_broadcast([P, JJ, 2])
            corrv = mp.tile([P, 32], f32)
            corrv3 = corrv[:].rearrange("p (j k) -> p j k", k=2)
            nc.vector.scalar_tensor_tensor(
                out=corrv3, in0=scn[:].rearrange("p (j k) -> p j k", k=2),
                scalar=1.0, in1=wvb,
                op0=Op.mult, op1=Op.mult, accum_out=stats[:, 3:4])

            # ---- ACT per-GT round 2: obj pair first (btob is ready before
            #      b), then the coord lns, then the class ln
            eo = mp.tile([P, JJ], f32)
            nc.scalar.activation(out=eo[:], in_=btob[:], func=Act.Exp, scale=-1.0)
            so = mp.tile([P, JJ], f32)
            nc.scalar.activation(out=so[:], in_=eo[:], func=Act.Ln, bias=1.0)
            lnp = mp.tile([P, 32], f32)
            lnp3 = lnp[:].rearrange("p (j c) -> p j c", c=2)
            nc.scalar.activation(
                out=lnp3, in_=b3[:, :, 2:4], func=Act.Ln,
                bias=meta[:, M_EPS:M_EPS + 1])
            syp = mp.tile([P, 32], f32)
            syp_i = nc.scalar.activation(
                out=syp[:], in_=lnp[:], func=Act.Exp, scale=0.5)
            ls = mp.tile([P, JJ], f32)
            nc.scalar.activation(
                out=ls[:], in_=sm[:], func=Act.Ln,
                bias=meta[:, M_EPS:M_EPS + 1])

            # ---- ACT stream pass 2: ln of the level-2 products, accumulated;
            #      pinned after the (latency-critical) round-2 activations
            lnm = mp.tile([P, QRT], f32)
            lnm_i = nc.scalar.activation(
                out=lnm[:], in_=m2[:], func=Act.Ln,
                accum_out=stats[:, 4:5])
            add_dep_helper(lnm_i.ins, syp_i.ins, False,
                           "stream ln after per-GT round 2")

            # ---- DVE tail: coord/obj/class accumulations
            dwh = mp.tile([P, 32], f32)
            dwh3 = dwh[:].rearrange("p (j c) -> p j c", c=2)
            nc.vector.tensor_tensor(
                dwh3, syp[:].rearrange("p (j c) -> p j c", c=2), syg3,
                op=Op.subtract)
            nc.vector.tensor_tensor(d24[:, :, 2:4], dwh3, dwh3, op=Op.mult)
            gtvb = gtv.unsqueeze(2).to_broadcast([P, JJ, 4])
            coordv = mp.tile([P, 64], f32)
            coordv3 = coordv[:].rearrange("p (j m) -> p j m", m=4)
            nc.vector.scalar_tensor_tensor(
                out=coordv3, in0=d24, scalar=1.0, in1=gtvb,
                op0=Op.mult, op1=Op.mult, accum_out=stats[:, 0:1])
            objv = mp.tile([P, JJ], f32)
            nc.vector.scalar_tensor_tensor(
                out=objv[:], in0=so[:], scalar=1.0, in1=gtv,
                op0=Op.mult, op1=Op.mult, accum_out=stats[:, 1:2])
            lsv = mp.tile([P, JJ], f32)
            nc.vector.scalar_tensor_tensor(
                out=lsv[:], in0=ls[:], scalar=1.0, in1=gtv,
                op0=Op.mult, op1=Op.mult, accum_out=stats[:, 2:3])

            # ---- cross-partition reduce: ones^T @ stats
            ps = pp.tile([1, 8], f32)
            nc.tensor.matmul(
                out=ps[:], lhsT=meta[:, M_ONE:M_ONE + 1], rhs=stats[:],
                start=True, stop=True)
            outt = mp.tile([1, 8], f32)
            nc.vector.tensor_copy(out=outt[:], in_=ps[:])
            nc.sync.dma_start(out=out_d[:], in_=outt[:])

    nc.compile()
    return nc


_NC_CACHE = {}


def _get_program(for_sim: bool = False) -> bass.Bass:
    key = bool(for_sim)
    if key not in _NC_CACHE:
        _NC_CACHE[key] = build_program(for_sim)
    return _NC_CACHE[key]


def make_in_maps(predictions, gt_boxes, gt_labels, gt_valid):
    predictions = np.ascontiguousarray(np.asarray(predictions), np.float32)
    gtb = np.ascontiguousarray(np.asarray(gt_boxes), np.float32)
    gtl = np.asarray(gt_labels).astype(np.int64)
    gtv = np.asarray(gt_valid).astype(bool)
    f52 = np.float32(S)
    in_maps = []
    for c in range(NCORES):
        sl = slice(c * NIMG, (c + 1) * NIMG)
        pred = predictions[sl].reshape(ROWS, 18)
        # compact objectness stream, fp8 e4m3
        obj = np.ascontiguousarray(pred[:, 0:10:5]).reshape(P, NOBJ)
        obj = obj.astype(ml_dtypes.float8_e4m3)

        b = gtb[sl].reshape(NG, 4)
        cx, cy, w, h = b[:, 0], b[:, 1], b[:, 2], b[:, 3]
        # same float32 ops the reference does: floor(clip) of cx*S / cy*S
        gj = np.clip(np.floor(cx * f52), 0, S - 1).astype(np.float32)
        gi = np.clip(np.floor(cy * f52), 0, S - 1).astype(np.float32)
        g = np.arange(NG)
        row = ((g // N_GT) * CELLS + gi.astype(np.int64) * S
               + gj.astype(np.int64))
        # host gather of the GT cells, channel-blocked j-major
        cg = pred[row]                                   # (NG, 18)

        v = gtv[sl].reshape(NG)
        # dedup: count each GT cell once per image (first valid GT wins)
        cell_img = row.reshape(NIMG, N_GT)
        vi = v.reshape(NIMG, N_GT)
        same = cell_img[:, :, None] == cell_img[:, None, :]   # (I, j, q)
        tri = np.tril(np.ones((N_GT, N_GT), bool), -1)        # q < j
        dup = (same & vi[:, None, :] & tri[None]).any(axis=2)
        wv = (vi & ~dup).reshape(NG).astype(np.float32)

        half = np.float32(0.5)
        g1x, g1y = cx - w * half, cy - h * half
        g2x, g2y = cx + w * half, cy + h * half
        a2e = ((g2x - g1x) * (g2y - g1y) + np.float32(EPS)).astype(np.float32)
        syw = np.sqrt(w + np.float32(EPS), dtype=np.float32)
        syh = np.sqrt(h + np.float32(EPS), dtype=np.float32)

        lab = gtl[sl].reshape(NG)
        oh = ((lab[:, None] == np.arange(NCLS)[None, :])
              & (v[:, None])).astype(np.float32)

        meta = np.zeros((P, MW), np.float32)
        meta[:, C_TXY:C_TXY + 4 * JJ] = -cg[:, [1, 2, 6, 7]].reshape(P, 4 * JJ)
        meta[:, C_TWH:C_TWH + 4 * JJ] = cg[:, [3, 4, 8, 9]].reshape(P, 4 * JJ)
        meta[:, M_ONE] = 1.0
        meta[:, M_EPS] = EPS
        meta[:, M_G1:M_G1 + 2 * JJ] = np.stack(
            [g1x, g1y], 1).reshape(P, 2 * JJ)
        meta[:, M_G2:M_G2 + 2 * JJ] = np.stack(
            [g2x, g2y], 1).reshape(P, 2 * JJ)
        meta[:, M_A2E:M_A2E + JJ] = a2e.reshape(P, JJ)
        meta[:, M_SYG:M_SYG + 2 * JJ] = np.stack(
            [syw, syh], 1).reshape(P, 2 * JJ)
        meta[:, M_GXY:M_GXY + 2 * JJ] = np.stack(
            [cx, cy], 1).reshape(P, 2 * JJ)

        metab = np.zeros((P, MBW), np.float32)
        metab[:, CB_CLS:CB_CLS + NCLS * JJ] = cg[:, 10:18].reshape(P, NCLS * JJ)
        metab[:, CB_OBJ:CB_OBJ + 2 * JJ] = cg[:, [0, 5]].reshape(P, 2 * JJ)
        metab[:, B_GJS:B_GJS + 2 * JJ] = np.stack(
            [gj, gi], 1).reshape(P, 2 * JJ)
        metab[:, B_WV:B_WV + JJ] = wv.reshape(P, JJ)
        metab[:, B_GTV:B_GTV + JJ] = v.astype(np.float32).reshape(P, JJ)
        metab[:, B_OH:B_OH + NCLS * JJ] = oh.reshape(P, NCLS * JJ)

        in_maps.append({
            "obj": np.ascontiguousarray(obj),
            "mf": np.ascontiguousarray(meta),
            "mb": np.ascontiguousarray(metab.astype(ml_dtypes.bfloat16)),
        })
    return in_maps


def combine_outputs(outs):
    """outs: list of (1, 8) per-core partials -> 5-tuple of scalars."""
    t = np.stack([np.asarray(o).reshape(8) for o in outs]).astype(np.float64)
    s = t.sum(0)
    coord, obj, corr, stream = s[0], s[1], s[3], s[4]
    cls = s[2] - s[5]
    noobj = stream - corr
    total = (LAMBDA_COORD * coord + obj + LAMBDA_NOOBJ * noobj + cls) / BATCH
    return (np.float32(total), np.float32(coord / BATCH),
            np.float32(obj / BATCH), np.float32(noobj / BATCH),
            np.float32(cls / BATCH))


def kernel(predictions, gt_boxes, gt_labels, gt_valid):
    from concourse.bass_utils import run_bass_kernel_spmd

    nc = _get_program(for_sim=False)
    in_maps = make_in_maps(predictions, gt_boxes, gt_labels, gt_valid)
    try:
        res = run_bass_kernel_spmd(nc, in_maps, list(range(NCORES))).results
    except Exception:
        # transient NRT_EXEC_UNIT_UNRECOVERABLE has been observed right
        # after an earlier crashed run; one retry clears it
        res = run_bass_kernel_spmd(nc, in_maps, list(range(NCORES))).results
    return combine_outputs([r["out"] for r in res])


# revision 53
# speedup vs baseline: 1.0055x; 1.0055x over previous
"""Trainium2 Bass kernel for YOLO-style DetectionLoss.

Contract: kernel(**inputs) takes the FULL inputs (batch 512) and returns the
full output (5-tuple of f32 scalars), sharding batch-wise across 8 NeuronCores.

Device-side strategy (per core: 64 images, 2048 GTs):
  - the noobj term needs only channels {0,5} (objectness logits) of every
    cell; they ship as a compact fp8 stream [128, 2704] (346 KB vs the
    12.5 MB full f32 shard), softplus'd on ACT as ln(1+e^x) with the ln
    pass shrunk 4x by a bf16 pairwise product tree on DVE:
    sum ln(1+e^x) = sum ln(prod_4 (1+e^x))
  - the 2048 GT cells are host-gathered into channel-blocked packs (txy,
    twh in f32 -- they decide the responsible-box argmax ties -- cls, obj
    in bf16) merged with the gt meta by dtype, so the device pays three
    DMAs total instead of 16 indirect row-gathers, and the decode exps
    are contiguous ACT instructions
  - per-GT work (sigmoid decode, IoU responsible-box pick, coord/obj/class
    losses, noobj dedup correction) runs on DVE/ACT mirroring the
    reference's math; the IoU compare uses i1*A0 > i0*A1 (the i0*i1 terms
    of the cross-multiplied union compare cancel)
  - gt-derived bookkeeping (cell indices, corner boxes, sqrt targets,
    first-GT-in-cell dedup mask, validity-masked one-hot labels) is
    host-precomputed from the small gt tensors; exact-in-bf16 parts ship
    as bf16
  - accumulators land in one [128, 8] stats tile via accum_out (DVE cols
    0-3,5; ACT col 4), reduced across partitions with a ones-vector
    matmul; the host sums the 8 per-core rows.

Scheduling notes: Tile orders each engine's stream from its own cost
estimates, so two dep edges pin the critical path -- the stream pairing
(q) behind the responsible-box chain on DVE, and the big stream ln (lnm)
behind round-2 on ACT. DMAs split across the sync and scalar HWDGE rings;
the gpsimd SWDGE path measured ~54 GB/s and is avoided entirely.
"""
import sys

sys.path.insert(0, "/opt/trn_rl_repo")

import numpy as np
import ml_dtypes

import concourse.bass as bass
import concourse.tile as tile
from concourse import bacc, mybir
from concourse.tile import add_dep_helper

S = 52
NBOX = 2
NCLS = 8
EPS = 1e-6
LAMBDA_COORD = 5.0
LAMBDA_NOOBJ = 0.5
BATCH = 512
N_GT = 32
NCORES = 8
NIMG = BATCH // NCORES          # 64 images per core
CELLS = S * S                   # 2704
ROWS = NIMG * CELLS             # 173056 cells per core
NG = NIMG * N_GT                # 2048 GTs per core
P = 128
JJ = NG // P                    # 16 GTs per partition
NOBJ = ROWS * NBOX // P         # 2704 obj logits per partition
HALF = NOBJ // 2                # 1352
QRT = NOBJ // 4                 # 676

f32 = mybir.dt.float32
bf16 = mybir.dt.bfloat16
f8 = mybir.dt.float8e4
Act = mybir.ActivationFunctionType
Op = mybir.AluOpType
AxX = mybir.AxisListType.X

# mf (f32, [P, 280]): gathered-cell txy/twh + f32 gt meta, one DMA
C_TXY = 0                       # 64: (j,k,c) j-major
C_TWH = 64                      # 64
M_ONE = 128                     # ones (matmul reduce vector)
M_EPS = 129                     # EPS (activation bias AP)
M_G1 = 136                      # 32: gt corner lo (x,y), j-major
M_G2 = 168                      # 32: gt corner hi (x,y), j-major
M_A2E = 200                     # 16: gt area + EPS
M_SYG = 216                     # 32: (sqrt(w+eps), sqrt(h+eps)), j-major
M_GXY = 248                     # 32: (cx, cy), j-major
MW = 280

# mb (bf16, [P, 352]): gathered-cell cls/obj + bf16-exact gt meta, one DMA
CB_CLS = 0                      # 128: (j,c) j-major
CB_OBJ = 128                    # 32: (j,k)
B_GJS = 160                     # 32: (gj, gi), j-major
B_WV = 192                      # 16: valid & first-GT-in-cell dedup weight
B_GTV = 208                     # 16: gt valid
B_OH = 224                      # 128: validity-masked class one-hot
MBW = 352

_ACT_PATCHED = False


def _force_single_act_table():
    """Place every activation in natural_log_exp_and_others (covers Exp+Ln)
    so the kernel pays one ACT_TABLE_LOAD."""
    global _ACT_PATCHED
    if _ACT_PATCHED:
        return
    from concourse import hw_specs

    orig = hw_specs.get_activation_tables

    def patched(arch):
        t = orig(arch)
        keep = "natural_log_exp_and_others"
        if keep not in t:
            return t
        return {k: (v if k == keep else set()) for k, v in t.items()}

    hw_specs.get_activation_tables = patched
    bacc.get_activation_tables = patched
    _ACT_PATCHED = True


def build_program(for_sim: bool = False) -> bass.Bass:
    _force_single_act_table()
    nc = bacc.Bacc(None, target_bir_lowering=False)

    mf_d = nc.dram_tensor("mf", [P, MW], f32, kind="ExternalInput")
    mb_d = nc.dram_tensor("mb", [P, MBW], bf16, kind="ExternalInput")
    obj_d = nc.dram_tensor("obj", [P, NOBJ], f8, kind="ExternalInput")
    out_d = nc.dram_tensor("out", [1, 8], f32, kind="ExternalOutput")

    with tile.TileContext(nc) as tc:
        with (
            tc.tile_pool(name="main", bufs=1) as mp,
            tc.tile_pool(name="psum", bufs=1, space="PSUM") as pp,
        ):
            stats = mp.tile([P, 8], f32)

            # ---- DMA: the chain-gating txy/twh half of the f32 pack leads
            #      the sync ring (the decode exp and the whole DVE chain
            #      hang off it), the fp8 stream behind it; the bf16 pack
            #      (gjs feeds the chain early) then the rest of the f32 pack
            #      (corners, needed ~1.5us into the chain) on the scalar ring
            meta = mp.tile([P, MW], f32)
            nc.sync.dma_start(out=meta[:, 0:128], in_=mf_d[:, 0:128])
            objt = mp.tile([P, NOBJ], f8)
            nc.sync.dma_start(out=objt[:], in_=obj_d[:])
            metab = mp.tile([P, MBW], bf16)
            nc.scalar.dma_start(out=metab[:], in_=mb_d[:])
            nc.scalar.dma_start(out=meta[:, 128:MW], in_=mf_d[:, 128:MW])
            cells = meta
            cellsb = metab

            # meta views
            g13 = meta[:, M_G1:M_G1 + 2 * JJ].rearrange("p (j c) -> p j c", c=2)
            g23 = meta[:, M_G2:M_G2 + 2 * JJ].rearrange("p (j c) -> p j c", c=2)
            a2e = meta[:, M_A2E:M_A2E + JJ]
            syg3 = meta[:, M_SYG:M_SYG + 2 * JJ].rearrange(
                "p (j c) -> p j c", c=2)
            gxy3 = meta[:, M_GXY:M_GXY + 2 * JJ].rearrange(
                "p (j c) -> p j c", c=2)
            gjs3 = metab[:, B_GJS:B_GJS + 2 * JJ].rearrange(
                "p (j c) -> p j c", c=2)
            wv = metab[:, B_WV:B_WV + JJ]
            gtv = metab[:, B_GTV:B_GTV + JJ]
            oh3 = metab[:, B_OH:B_OH + NCLS * JJ].rearrange(
                "p (j c) -> p j c", c=NCLS)

            # cells views
            cls3 = cellsb[:, CB_CLS:CB_CLS + 128].rearrange(
                "p (j c) -> p j c", c=NCLS)
            cobj3 = cellsb[:, CB_OBJ:CB_OBJ + 32].rearrange(
                "p (j k) -> p j k", k=2)

            # ---- ACT: decode exps first (they gate the long DVE chain).
            #      txy ships negated in mf, so one Exp over [P,128] yields
            #      e^{-txy} | e^{twh} together.
            exw = mp.tile([P, 128], f32)   # e^{-txy}(64) | e^{twh}(64)
            nc.scalar.activation(
                out=exw[:], in_=cells[:, 0:128], func=Act.Exp)
            exy = exw[:, 0:64]
            ewh = exw[:, 64:128]
            eco = mp.tile([P, 160], f32)   # e^{cls}(128) | e^{obj}(32)
            nc.scalar.activation(out=eco[:], in_=cellsb[:, 0:160], func=Act.Exp)
            scn = mp.tile([P, 32], f32)    # softplus(obj logits) at GT cells
            nc.scalar.activation(
                out=scn[:], in_=eco[:, 128:160], func=Act.Ln, bias=1.0)
            et = mp.tile([P, NOBJ], bf16)
            nc.scalar.activation(out=et[:], in_=objt[:], func=Act.Exp)

            ecls3 = eco[:, 0:128].rearrange("p (j c) -> p j c", c=NCLS)

            # ---- DVE per-GT mid chain: decode, IoU, responsible pick
            den = mp.tile([P, 64], f32)
            nc.vector.tensor_scalar(den[:], exy, 1.0, None, Op.add)
            sgm = mp.tile([P, 64], f32)
            # den = 1+e^{-t} in (1, ~250): no edge cases; ~51 ULP is far
            # below the responsible-box tie noise, ~5x faster than reciprocal
            nc.vector.reciprocal_approx_fast(out=sgm[:], in_=den[:])
            sgm4 = sgm[:].rearrange("p (j k c) -> p j k c", k=2, c=2)
            pb = mp.tile([P, 128], f32)
            pb4 = pb[:].rearrange("p (j k m) -> p j k m", k=2, m=4)
            pbv = pb[:].rearrange("p (t m) -> p t m", m=4)
            # px = (sigmoid + gj) * (1/S), matching the reference's order
            gjb = gjs3.unsqueeze(2).to_broadcast([P, JJ, 2, 2])
            sgp = mp.tile([P, 64], f32)
            sgp4 = sgp[:].rearrange("p (j k c) -> p j k c", k=2, c=2)
            nc.vector.tensor_tensor(sgp4, sgm4, gjb, op=Op.add)
            nc.vector.tensor_scalar(
                pbv[:, :, 0:2], sgp[:].rearrange("p (t c) -> p t c", c=2),
                1.0 / S, None, Op.mult)
            nc.vector.tensor_scalar(
                pbv[:, :, 2:4], ewh.rearrange("p (t c) -> p t c", c=2),
                1.0, None, Op.min)
            p1 = mp.tile([P, 64], f32)
            p14 = p1[:].rearrange("p (j k c) -> p j k c", k=2, c=2)
            nc.vector.scalar_tensor_tensor(
                out=p1[:].rearrange("p (t c) -> p t c", c=2),
                in0=pbv[:, :, 2:4], scalar=-0.5,
                in1=pbv[:, :, 0:2], op0=Op.mult, op1=Op.add)
            p2 = mp.tile([P, 64], f32)
            p24 = p2[:].rearrange("p (j k c) -> p j k c", k=2, c=2)
            nc.vector.scalar_tensor_tensor(
                out=p2[:].rearrange("p (t c) -> p t c", c=2),
                in0=pbv[:, :, 2:4], scalar=0.5,
                in1=pbv[:, :, 0:2], op0=Op.mult, op1=Op.add)
            g1b = g13.unsqueeze(2).to_broadcast([P, JJ, 2, 2])
            g2b = g23.unsqueeze(2).to_broadcast([P, JJ, 2, 2])
            lo = mp.tile([P, 64], f32)
            lo4 = lo[:].rearrange("p (j k c) -> p j k c", k=2, c=2)
            nc.vector.tensor_tensor(lo4, p14, g1b, op=Op.max)
            hi = mp.tile([P, 64], f32)
            hi4 = hi[:].rearrange("p (j k c) -> p j k c", k=2, c=2)
            nc.vector.tensor_tensor(hi4, p24, g2b, op=Op.min)
            iwr = mp.tile([P, 64], f32)
            nc.vector.tensor_tensor(iwr[:], hi[:], lo[:], op=Op.subtract)
            iwh = mp.tile([P, 64], f32)
            nc.vector.tensor_scalar(iwh[:], iwr[:], 0.0, None, Op.max)
            iwh4 = iwh[:].rearrange("p (j k c) -> p j k c", k=2, c=2)
            inter = mp.tile([P, 32], f32)
            inter3 = inter[:].rearrange("p (j k) -> p j k", k=2)
            nc.vector.tensor_tensor(
                inter3, iwh4[:, :, :, 0], iwh4[:, :, :, 1], op=Op.mult)
            a1 = mp.tile([P, 32], f32)
            a13 = a1[:].rearrange("p (j k) -> p j k", k=2)
            nc.vector.tensor_tensor(
                a13, pb4[:, :, :, 2], pb4[:, :, :, 3], op=Op.mult)
            # iou1 > iou0  <=>  i1*(A0-i0) > i0*(A1-i1)  <=>  i1*A0 > i0*A1
            # (A_k = area_k + area_gt + EPS; the i0*i1 terms cancel)
            a2b = a2e.unsqueeze(2).to_broadcast([P, JJ, 2])
            u1 = mp.tile([P, 32], f32)
            u13 = u1[:].rearrange("p (j k) -> p j k", k=2)
            nc.vector.tensor_tensor(u13, a13, a2b, op=Op.add)
            d0 = mp.tile([P, JJ], f32)
            nc.vector.tensor_tensor(
                d0[:], inter3[:, :, 0], u13[:, :, 1], op=Op.mult)
            d1 = mp.tile([P, JJ], f32)
            nc.vector.tensor_tensor(
                d1[:], inter3[:, :, 1], u13[:, :, 0], op=Op.mult)
            sel = mp.tile([P, JJ], f32)
            nc.vector.tensor_tensor(sel[:], d1[:], d0[:], op=Op.is_gt)
            # responsible obj logit first: it unblocks ACT's eo/so pair
            od = mp.tile([P, JJ], f32)
            nc.vector.tensor_tensor(
                od[:], cobj3[:, :, 1], cobj3[:, :, 0], op=Op.subtract)
            om = mp.tile([P, JJ], f32)
            nc.vector.tensor_tensor(om[:], od[:], sel[:], op=Op.mult)
            btob = mp.tile([P, JJ], f32)
            btob_i = nc.vector.tensor_tensor(
                btob[:], om[:], cobj3[:, :, 0], op=Op.add)
            selb4 = sel[:].unsqueeze(2).to_broadcast([P, JJ, 4])
            bd = mp.tile([P, 64], f32)
            bd3 = bd[:].rearrange("p (j m) -> p j m", m=4)
            nc.vector.tensor_tensor(
                bd3, pb4[:, :, 1, :], pb4[:, :, 0, :], op=Op.subtract)
            bm = mp.tile([P, 64], f32)
            bm3 = bm[:].rearrange("p (j m) -> p j m", m=4)
            nc.vector.tensor_tensor(bm3, bd3, selb4, op=Op.mult)
            b = mp.tile([P, 64], f32)
            b3 = b[:].rearrange("p (j m) -> p j m", m=4)
            nc.vector.tensor_tensor(b3, bm3, pb4[:, :, 0, :], op=Op.add)

            # coord xy part into packed d2 tile (j, [dx dy dw dh])
            d2 = mp.tile([P, 64], f32)
            d24 = d2[:].rearrange("p (j m) -> p j m", m=4)
            dxy = mp.tile([P, 32], f32)
            dxy3 = dxy[:].rearrange("p (j c) -> p j c", c=2)
            nc.vector.tensor_tensor(
                dxy3, b3[:, :, 0:2], gxy3, op=Op.subtract)
            nc.vector.tensor_tensor(d24[:, :, 0:2], dxy3, dxy3, op=Op.mult)

            # stream pairing (bf16 2x): q = 1 + e^x, two pairing levels so
            # the ACT ln pass covers NOBJ/4 elements; pinned behind btob so
            # it cannot preempt the latency-critical chain
            q = mp.tile([P, NOBJ], bf16)
            q_i = nc.vector.tensor_scalar(q[:], et[:], 1.0, None, Op.add)
            add_dep_helper(q_i.ins, btob_i.ins, False,
                           "stream pairing after the responsible-box chain")
            m1 = mp.tile([P, HALF], bf16)
            nc.vector.tensor_tensor(
                m1[:], q[:, 0:HALF], q[:, HALF:NOBJ], op=Op.mult)
            m2 = mp.tile([P, QRT], bf16)
            nc.vector.tensor_tensor(
                m2[:], m1[:, 0:QRT], m1[:, QRT:HALF], op=Op.mult)
            # class sums: cls_loss = sum(ls*v) - sum(cls*onehot_masked)
            sm = mp.tile([P, JJ], f32)
            nc.vector.tensor_reduce(sm[:], ecls3, axis=AxX, op=Op.add)
            pickv = mp.tile([P, NCLS * JJ], f32)
            pickv3 = pickv[:].rearrange("p (j c) -> p j c", c=NCLS)
            nc.vector.scalar_tensor_tensor(
                out=pickv3, in0=oh3, scalar=1.0, in1=cls3,
                op0=Op.mult, op1=Op.mult, accum_out=stats[:, 5:6])
            # noobj dedup correction
            wvb = wv.unsqueeze(2).to_broadcast([P, JJ, 2])
            corrv = mp.tile([P, 32], f32)
            corrv3 = corrv[:].rearrange("p (j k) -> p j k", k=2)
            nc.vector.scalar_tensor_tensor(
                out=corrv3, in0=scn[:].rearrange("p (j k) -> p j k", k=2),
                scalar=1.0, in1=wvb,
                op0=Op.mult, op1=Op.mult, accum_out=stats[:, 3:4])

            # ---- ACT per-GT round 2: obj pair first (btob is ready before
            #      b), then the coord lns, then the class ln
            eo = mp.tile([P, JJ], f32)
            nc.scalar.activation(out=eo[:], in_=btob[:], func=Act.Exp, scale=-1.0)
            so = mp.tile([P, JJ], f32)
            nc.scalar.activation(out=so[:], in_=eo[:], func=Act.Ln, bias=1.0)
            lnp = mp.tile([P, 32], f32)
            lnp3 = lnp[:].rearrange("p (j c) -> p j c", c=2)
            nc.scalar.activation(
                out=lnp3, in_=b3[:, :, 2:4], func=Act.Ln,
                bias=meta[:, M_EPS:M_EPS + 1])
            syp = mp.tile([P, 32], f32)
            syp_i = nc.scalar.activation(
                out=syp[:], in_=lnp[:], func=Act.Exp, scale=0.5)
            ls = mp.tile([P, JJ], f32)
            nc.scalar.activation(
                out=ls[:], in_=sm[:], func=Act.Ln,
                bias=meta[:, M_EPS:M_EPS + 1])

            # ---- ACT stream pass 2: ln of the level-2 products, accumulated;
            #      pinned after the (latency-critical) round-2 activations
            lnm = mp.tile([P, QRT], f32)
            lnm_i = nc.scalar.activation(
                out=lnm[:], in_=m2[:], func=Act.Ln,
                accum_out=stats[:, 4:5])
            add_dep_helper(lnm_i.ins, syp_i.ins, False,
                           "stream ln after per-GT round 2")

            # ---- DVE tail: coord/obj/class accumulations
            dwh = mp.tile([P, 32], f32)
            dwh3 = dwh[:].rearrange("p (j c) -> p j c", c=2)
            nc.vector.tensor_tensor(
                dwh3, syp[:].rearrange("p (j c) -> p j c", c=2), syg3,
                op=Op.subtract)
            nc.vector.tensor_tensor(d24[:, :, 2:4], dwh3, dwh3, op=Op.mult)
            gtvb = gtv.unsqueeze(2).to_broadcast([P, JJ, 4])
            coordv = mp.tile([P, 64], f32)
            coordv3 = coordv[:].rearrange("p (j m) -> p j m", m=4)
            nc.vector.scalar_tensor_tensor(
                out=coordv3, in0=d24, scalar=1.0, in1=gtvb,
                op0=Op.mult, op1=Op.mult, accum_out=stats[:, 0:1])
            objv = mp.tile([P, JJ], f32)
            nc.vector.scalar_tensor_tensor(
                out=objv[:], in0=so[:], scalar=1.0, in1=gtv,
                op0=Op.mult, op1=Op.mult, accum_out=stats[:, 1:2])
            lsv = mp.tile([P, JJ], f32)
            nc.vector.scalar_tensor_tensor(
                out=lsv[:], in0=ls[:], scalar=1.0, in1=gtv,
                op0=Op.mult, op1=Op.mult, accum_out=stats[:, 2:3])

            # ---- cross-partition reduce: ones^T @ stats
            ps = pp.tile([1, 8], f32)
            nc.tensor.matmul(
                out=ps[:], lhsT=meta[:, M_ONE:M_ONE + 1], rhs=stats[:],
                start=True, stop=True)
            outt = mp.tile([1, 8], f32)
            nc.vector.tensor_copy(out=outt[:], in_=ps[:])
            nc.sync.dma_start(out=out_d[:], in_=outt[:])

    nc.compile()
    return nc


_NC_CACHE = {}


def _get_program(for_sim: bool = False) -> bass.Bass:
    key = bool(for_sim)
    if key not in _NC_CACHE:
        _NC_CACHE[key] = build_program(for_sim)
    return _NC_CACHE[key]


def make_in_maps(predictions, gt_boxes, gt_labels, gt_valid):
    predictions = np.ascontiguousarray(np.asarray(predictions), np.float32)
    gtb = np.ascontiguousarray(np.asarray(gt_boxes), np.float32)
    gtl = np.asarray(gt_labels).astype(np.int64)
    gtv = np.asarray(gt_valid).astype(bool)
    f52 = np.float32(S)
    in_maps = []
    for c in range(NCORES):
        sl = slice(c * NIMG, (c + 1) * NIMG)
        pred = predictions[sl].reshape(ROWS, 18)
        # compact objectness stream, fp8 e4m3
        obj = np.ascontiguousarray(pred[:, 0:10:5]).reshape(P, NOBJ)
        obj = obj.astype(ml_dtypes.float8_e4m3)

        b = gtb[sl].reshape(NG, 4)
        cx, cy, w, h = b[:, 0], b[:, 1], b[:, 2], b[:, 3]
        # same float32 ops the reference does: floor(clip) of cx*S / cy*S
        gj = np.clip(np.floor(cx * f52), 0, S - 1).astype(np.float32)
        gi = np.clip(np.floor(cy * f52), 0, S - 1).astype(np.float32)
        g = np.arange(NG)
        row = ((g // N_GT) * CELLS + gi.astype(np.int64) * S
               + gj.astype(np.int64))
        # host gather of the GT cells, channel-blocked j-major
        cg = pred[row]                                   # (NG, 18)

        v = gtv[sl].reshape(NG)
        # dedup: count each GT cell once per image (first valid GT wins)
        cell_img = row.reshape(NIMG, N_GT)
        vi = v.reshape(NIMG, N_GT)
        same = cell_img[:, :, None] == cell_img[:, None, :]   # (I, j, q)
        tri = np.tril(np.ones((N_GT, N_GT), bool), -1)        # q < j
        dup = (same & vi[:, None, :] & tri[None]).any(axis=2)
        wv = (vi & ~dup).reshape(NG).astype(np.float32)

        half = np.float32(0.5)
        g1x, g1y = cx - w * half, cy - h * half
        g2x, g2y = cx + w * half, cy + h * half
        a2e = ((g2x - g1x) * (g2y - g1y) + np.float32(EPS)).astype(np.float32)
        syw = np.sqrt(w + np.float32(EPS), dtype=np.float32)
        syh = np.sqrt(h + np.float32(EPS), dtype=np.float32)

        lab = gtl[sl].reshape(NG)
        oh = ((lab[:, None] == np.arange(NCLS)[None, :])
              & (v[:, None])).astype(np.float32)

        meta = np.zeros((P, MW), np.float32)
        meta[:, C_TXY:C_TXY + 4 * JJ] = -cg[:, [1, 2, 6, 7]].reshape(P, 4 * JJ)
        meta[:, C_TWH:C_TWH + 4 * JJ] = cg[:, [3, 4, 8, 9]].reshape(P, 4 * JJ)
        meta[:, M_ONE] = 1.0
        meta[:, M_EPS] = EPS
        meta[:, M_G1:M_G1 + 2 * JJ] = np.stack(
            [g1x, g1y], 1).reshape(P, 2 * JJ)
        meta[:, M_G2:M_G2 + 2 * JJ] = np.stack(
            [g2x, g2y], 1).reshape(P, 2 * JJ)
        meta[:, M_A2E:M_A2E + JJ] = a2e.reshape(P, JJ)
        meta[:, M_SYG:M_SYG + 2 * JJ] = np.stack(
            [syw, syh], 1).reshape(P, 2 * JJ)
        meta[:, M_GXY:M_GXY + 2 * JJ] = np.stack(
            [cx, cy], 1).reshape(P, 2 * JJ)

        metab = np.zeros((P, MBW), np.float32)
        metab[:, CB_CLS:CB_CLS + NCLS * JJ] = cg[:, 10:18].reshape(P, NCLS * JJ)
        metab[:, CB_OBJ:CB_OBJ + 2 * JJ] = cg[:, [0, 5]].reshape(P, 2 * JJ)
        metab[:, B_GJS:B_GJS + 2 * JJ] = np.stack(
            [gj, gi], 1).reshape(P, 2 * JJ)
        metab[:, B_WV:B_WV + JJ] = wv.reshape(P, JJ)
        metab[:, B_GTV:B_GTV + JJ] = v.astype(np.float32).reshape(P, JJ)
        metab[:, B_OH:B_OH + NCLS * JJ] = oh.reshape(P, NCLS * JJ)

        in_maps.append({
            "obj": np.ascontiguousarray(obj),
            "mf": np.ascontiguousarray(meta),
            "mb": np.ascontiguousarray(metab.astype(ml_dtypes.bfloat16)),
        })
    return in_maps


def combine_outputs(outs):
    """outs: list of (1, 8) per-core partials -> 5-tuple of scalars."""
    t = np.stack([np.asarray(o).reshape(8) for o in outs]).astype(np.float64)
    s = t.sum(0)
    coord, obj, corr, stream = s[0], s[1], s[3], s[4]
    cls = s[2] - s[5]
    noobj = stream - corr
    total = (LAMBDA_COORD * coord + obj + LAMBDA_NOOBJ * noobj + cls) / BATCH
    return (np.float32(total), np.float32(coord / BATCH),
            np.float32(obj / BATCH), np.float32(noobj / BATCH),
            np.float32(cls / BATCH))


def kernel(predictions, gt_boxes, gt_labels, gt_valid):
    from concourse.bass_utils import run_bass_kernel_spmd

    nc = _get_program(for_sim=False)
    in_maps = make_in_maps(predictions, gt_boxes, gt_labels, gt_valid)
    try:
        res = run_bass_kernel_spmd(nc, in_maps, list(range(NCORES))).results
    except Exception:
        # transient NRT_EXEC_UNIT_UNRECOVERABLE has been observed right
        # after an earlier crashed run; one retry clears it
        res = run_bass_kernel_spmd(nc, in_maps, list(range(NCORES))).results
    return combine_outputs([r["out"] for r in res])


# revision 54
# speedup vs baseline: 1.0144x; 1.0088x over previous
"""Trainium2 Bass kernel for YOLO-style DetectionLoss.

Contract: kernel(**inputs) takes the FULL inputs (batch 512) and returns the
full output (5-tuple of f32 scalars), sharding batch-wise across 8 NeuronCores.

Device-side strategy (per core: 64 images, 2048 GTs):
  - the noobj term needs only channels {0,5} (objectness logits) of every
    cell; they ship as a compact fp8 stream [128, 2704] (346 KB vs the
    12.5 MB full f32 shard), softplus'd on ACT as ln(1+e^x) with the ln
    pass shrunk 4x by a bf16 pairwise product tree on DVE:
    sum ln(1+e^x) = sum ln(prod_4 (1+e^x))
  - the 2048 GT cells are host-gathered into channel-blocked packs (txy,
    twh in f32 -- they decide the responsible-box argmax ties -- cls, obj
    in bf16) merged with the gt meta by dtype, so the device pays three
    DMAs total instead of 16 indirect row-gathers, and the decode exps
    are contiguous ACT instructions
  - per-GT work (sigmoid decode, IoU responsible-box pick, coord/obj/class
    losses, noobj dedup correction) runs on DVE/ACT mirroring the
    reference's math; the IoU compare uses i1*A0 > i0*A1 (the i0*i1 terms
    of the cross-multiplied union compare cancel)
  - gt-derived bookkeeping (cell indices, corner boxes, sqrt targets,
    first-GT-in-cell dedup mask, validity-masked one-hot labels) is
    host-precomputed from the small gt tensors; exact-in-bf16 parts ship
    as bf16
  - accumulators land in one [128, 8] stats tile via accum_out (DVE cols
    0-3,5; ACT col 4), reduced across partitions with a ones-vector
    matmul; the host sums the 8 per-core rows.

Scheduling notes: Tile orders each engine's stream from its own cost
estimates, so two dep edges pin the critical path -- the stream pairing
(q) behind the responsible-box chain on DVE, and the big stream ln (lnm)
behind round-2 on ACT. DMAs split across the sync and scalar HWDGE rings;
the gpsimd SWDGE path measured ~54 GB/s and is avoided entirely.
"""
import sys

sys.path.insert(0, "/opt/trn_rl_repo")

import numpy as np
import ml_dtypes

import concourse.bass as bass
import concourse.tile as tile
from concourse import bacc, mybir
from concourse.tile import add_dep_helper

S = 52
NBOX = 2
NCLS = 8
EPS = 1e-6
LAMBDA_COORD = 5.0
LAMBDA_NOOBJ = 0.5
BATCH = 512
N_GT = 32
NCORES = 8
NIMG = BATCH // NCORES          # 64 images per core
CELLS = S * S                   # 2704
ROWS = NIMG * CELLS             # 173056 cells per core
NG = NIMG * N_GT                # 2048 GTs per core
P = 128
JJ = NG // P                    # 16 GTs per partition
NOBJ = ROWS * NBOX // P         # 2704 obj logits per partition
HALF = NOBJ // 2                # 1352
QRT = NOBJ // 4                 # 676

f32 = mybir.dt.float32
bf16 = mybir.dt.bfloat16
f8 = mybir.dt.float8e4
Act = mybir.ActivationFunctionType
Op = mybir.AluOpType
AxX = mybir.AxisListType.X

# mf (f32, [P, 280]): gathered-cell txy/twh + f32 gt meta, one DMA
C_TXY = 0                       # 64: (j,k,c) j-major
C_TWH = 64                      # 64
M_ONE = 128                     # ones (matmul reduce vector)
M_EPS = 129                     # EPS (activation bias AP)
M_G1 = 136                      # 32: gt corner lo (x,y), j-major
M_G2 = 168                      # 32: gt corner hi (x,y), j-major
M_A2E = 200                     # 16: gt area + EPS
M_SYG = 216                     # 32: (sqrt(w+eps), sqrt(h+eps)), j-major
M_GXY = 248                     # 32: (cx, cy), j-major
MW = 280

# mb (bf16, [P, 352]): gathered-cell cls/obj + bf16-exact gt meta, one DMA
CB_CLS = 0                      # 128: (j,c) j-major
CB_OBJ = 128                    # 32: (j,k)
B_GJS = 160                     # 32: (gj, gi), j-major
B_WV = 192                      # 16: valid & first-GT-in-cell dedup weight
B_GTV = 208                     # 16: gt valid
B_OH = 224                      # 128: validity-masked class one-hot
MBW = 352

_ACT_PATCHED = False


def _force_single_act_table():
    """Place every activation in natural_log_exp_and_others (covers Exp+Ln)
    so the kernel pays one ACT_TABLE_LOAD."""
    global _ACT_PATCHED
    if _ACT_PATCHED:
        return
    from concourse import hw_specs

    orig = hw_specs.get_activation_tables

    def patched(arch):
        t = orig(arch)
        keep = "natural_log_exp_and_others"
        if keep not in t:
            return t
        return {k: (v if k == keep else set()) for k, v in t.items()}

    hw_specs.get_activation_tables = patched
    bacc.get_activation_tables = patched
    _ACT_PATCHED = True


def build_program(for_sim: bool = False) -> bass.Bass:
    _force_single_act_table()
    nc = bacc.Bacc(None, target_bir_lowering=False)

    mf_d = nc.dram_tensor("mf", [P, MW], f32, kind="ExternalInput")
    mb_d = nc.dram_tensor("mb", [P, MBW], bf16, kind="ExternalInput")
    obj_d = nc.dram_tensor("obj", [P, NOBJ], f8, kind="ExternalInput")
    out_d = nc.dram_tensor("out", [1, 8], f32, kind="ExternalOutput")

    with tile.TileContext(nc) as tc:
        with (
            tc.tile_pool(name="main", bufs=1) as mp,
            tc.tile_pool(name="psum", bufs=1, space="PSUM") as pp,
        ):
            stats = mp.tile([P, 8], f32)

            # ---- DMA: the chain-gating txy/twh half of the f32 pack leads
            #      the sync ring (the decode exp and the whole DVE chain
            #      hang off it), the fp8 stream behind it; the bf16 pack
            #      (gjs feeds the chain early) then the rest of the f32 pack
            #      (corners, needed ~1.5us into the chain) on the scalar ring
            meta = mp.tile([P, MW], f32)
            nc.sync.dma_start(out=meta[:, 0:128], in_=mf_d[:, 0:128])
            objt = mp.tile([P, NOBJ], f8)
            nc.sync.dma_start(out=objt[:], in_=obj_d[:])
            metab = mp.tile([P, MBW], bf16)
            nc.scalar.dma_start(out=metab[:], in_=mb_d[:])
            nc.scalar.dma_start(out=meta[:, 128:MW], in_=mf_d[:, 128:MW])
            cells = meta
            cellsb = metab

            # meta views
            g13 = meta[:, M_G1:M_G1 + 2 * JJ].rearrange("p (j c) -> p j c", c=2)
            g23 = meta[:, M_G2:M_G2 + 2 * JJ].rearrange("p (j c) -> p j c", c=2)
            a2e = meta[:, M_A2E:M_A2E + JJ]
            syg3 = meta[:, M_SYG:M_SYG + 2 * JJ].rearrange(
                "p (j c) -> p j c", c=2)
            gxy3 = meta[:, M_GXY:M_GXY + 2 * JJ].rearrange(
                "p (j c) -> p j c", c=2)
            gjs3 = metab[:, B_GJS:B_GJS + 2 * JJ].rearrange(
                "p (j c) -> p j c", c=2)
            wv = metab[:, B_WV:B_WV + JJ]
            gtv = metab[:, B_GTV:B_GTV + JJ]
            oh3 = metab[:, B_OH:B_OH + NCLS * JJ].rearrange(
                "p (j c) -> p j c", c=NCLS)

            # cells views
            cls3 = cellsb[:, CB_CLS:CB_CLS + 128].rearrange(
                "p (j c) -> p j c", c=NCLS)
            cobj3 = cellsb[:, CB_OBJ:CB_OBJ + 32].rearrange(
                "p (j k) -> p j k", k=2)

            # ---- ACT: decode exps first (they gate the long DVE chain).
            #      txy ships negated in mf, so one Exp over [P,128] yields
            #      e^{-txy} | e^{twh} together.
            exw = mp.tile([P, 128], f32)   # e^{-txy}(64) | e^{twh}(64)
            nc.scalar.activation(
                out=exw[:], in_=cells[:, 0:128], func=Act.Exp)
            exy = exw[:, 0:64]
            ewh = exw[:, 64:128]
            eco = mp.tile([P, 160], f32)   # e^{cls}(128) | e^{obj}(32)
            nc.scalar.activation(out=eco[:], in_=cellsb[:, 0:160], func=Act.Exp)
            scn = mp.tile([P, 32], f32)    # softplus(obj logits) at GT cells
            nc.scalar.activation(
                out=scn[:], in_=eco[:, 128:160], func=Act.Ln, bias=1.0)
            et = mp.tile([P, NOBJ], bf16)
            nc.scalar.activation(out=et[:], in_=objt[:], func=Act.Exp)

            ecls3 = eco[:, 0:128].rearrange("p (j c) -> p j c", c=NCLS)

            # ---- DVE per-GT mid chain: decode, IoU, responsible pick
            den = mp.tile([P, 64], f32)
            nc.vector.tensor_scalar(den[:], exy, 1.0, None, Op.add)
            sgm = mp.tile([P, 64], f32)
            # den = 1+e^{-t} in (1, ~250): no edge cases; ~51 ULP is far
            # below the responsible-box tie noise, ~5x faster than reciprocal
            nc.vector.reciprocal_approx_fast(out=sgm[:], in_=den[:])
            sgm4 = sgm[:].rearrange("p (j k c) -> p j k c", k=2, c=2)
            pb = mp.tile([P, 128], f32)
            pb4 = pb[:].rearrange("p (j k m) -> p j k m", k=2, m=4)
            pbv = pb[:].rearrange("p (t m) -> p t m", m=4)
            # cell-local coordinates: gt corners/centers ship shifted by
            #      -gj/S on host (IoU and coord diffs are translation
            #      invariant), so px = sigmoid * (1/S) directly
            nc.vector.tensor_scalar(
                pbv[:, :, 0:2], sgm[:].rearrange("p (t c) -> p t c", c=2),
                1.0 / S, None, Op.mult)
            nc.vector.tensor_scalar(
                pbv[:, :, 2:4], ewh.rearrange("p (t c) -> p t c", c=2),
                1.0, None, Op.min)
            p1 = mp.tile([P, 64], f32)
            p14 = p1[:].rearrange("p (j k c) -> p j k c", k=2, c=2)
            nc.vector.scalar_tensor_tensor(
                out=p1[:].rearrange("p (t c) -> p t c", c=2),
                in0=pbv[:, :, 2:4], scalar=-0.5,
                in1=pbv[:, :, 0:2], op0=Op.mult, op1=Op.add)
            p2 = mp.tile([P, 64], f32)
            p24 = p2[:].rearrange("p (j k c) -> p j k c", k=2, c=2)
            nc.vector.scalar_tensor_tensor(
                out=p2[:].rearrange("p (t c) -> p t c", c=2),
                in0=pbv[:, :, 2:4], scalar=0.5,
                in1=pbv[:, :, 0:2], op0=Op.mult, op1=Op.add)
            g1b = g13.unsqueeze(2).to_broadcast([P, JJ, 2, 2])
            g2b = g23.unsqueeze(2).to_broadcast([P, JJ, 2, 2])
            lo = mp.tile([P, 64], f32)
            lo4 = lo[:].rearrange("p (j k c) -> p j k c", k=2, c=2)
            nc.vector.tensor_tensor(lo4, p14, g1b, op=Op.max)
            hi = mp.tile([P, 64], f32)
            hi4 = hi[:].rearrange("p (j k c) -> p j k c", k=2, c=2)
            nc.vector.tensor_tensor(hi4, p24, g2b, op=Op.min)
            iwr = mp.tile([P, 64], f32)
            nc.vector.tensor_tensor(iwr[:], hi[:], lo[:], op=Op.subtract)
            iwh = mp.tile([P, 64], f32)
            nc.vector.tensor_scalar(iwh[:], iwr[:], 0.0, None, Op.max)
            iwh4 = iwh[:].rearrange("p (j k c) -> p j k c", k=2, c=2)
            inter = mp.tile([P, 32], f32)
            inter3 = inter[:].rearrange("p (j k) -> p j k", k=2)
            nc.vector.tensor_tensor(
                inter3, iwh4[:, :, :, 0], iwh4[:, :, :, 1], op=Op.mult)
            a1 = mp.tile([P, 32], f32)
            a13 = a1[:].rearrange("p (j k) -> p j k", k=2)
            nc.vector.tensor_tensor(
                a13, pb4[:, :, :, 2], pb4[:, :, :, 3], op=Op.mult)
            # iou1 > iou0  <=>  i1*(A0-i0) > i0*(A1-i1)  <=>  i1*A0 > i0*A1
            # (A_k = area_k + area_gt + EPS; the i0*i1 terms cancel)
            a2b = a2e.unsqueeze(2).to_broadcast([P, JJ, 2])
            u1 = mp.tile([P, 32], f32)
            u13 = u1[:].rearrange("p (j k) -> p j k", k=2)
            nc.vector.tensor_tensor(u13, a13, a2b, op=Op.add)
            d0 = mp.tile([P, JJ], f32)
            nc.vector.tensor_tensor(
                d0[:], inter3[:, :, 0], u13[:, :, 1], op=Op.mult)
            d1 = mp.tile([P, JJ], f32)
            nc.vector.tensor_tensor(
                d1[:], inter3[:, :, 1], u13[:, :, 0], op=Op.mult)
            sel = mp.tile([P, JJ], f32)
            nc.vector.tensor_tensor(sel[:], d1[:], d0[:], op=Op.is_gt)
            # responsible obj logit first: it unblocks ACT's eo/so pair
            od = mp.tile([P, JJ], f32)
            nc.vector.tensor_tensor(
                od[:], cobj3[:, :, 1], cobj3[:, :, 0], op=Op.subtract)
            om = mp.tile([P, JJ], f32)
            nc.vector.tensor_tensor(om[:], od[:], sel[:], op=Op.mult)
            btob = mp.tile([P, JJ], f32)
            btob_i = nc.vector.tensor_tensor(
                btob[:], om[:], cobj3[:, :, 0], op=Op.add)
            selb4 = sel[:].unsqueeze(2).to_broadcast([P, JJ, 4])
            bd = mp.tile([P, 64], f32)
            bd3 = bd[:].rearrange("p (j m) -> p j m", m=4)
            nc.vector.tensor_tensor(
                bd3, pb4[:, :, 1, :], pb4[:, :, 0, :], op=Op.subtract)
            bm = mp.tile([P, 64], f32)
            bm3 = bm[:].rearrange("p (j m) -> p j m", m=4)
            nc.vector.tensor_tensor(bm3, bd3, selb4, op=Op.mult)
            b = mp.tile([P, 64], f32)
            b3 = b[:].rearrange("p (j m) -> p j m", m=4)
            nc.vector.tensor_tensor(b3, bm3, pb4[:, :, 0, :], op=Op.add)

            # coord xy part into packed d2 tile (j, [dx dy dw dh])
            d2 = mp.tile([P, 64], f32)
            d24 = d2[:].rearrange("p (j m) -> p j m", m=4)
            dxy = mp.tile([P, 32], f32)
            dxy3 = dxy[:].rearrange("p (j c) -> p j c", c=2)
            nc.vector.tensor_tensor(
                dxy3, b3[:, :, 0:2], gxy3, op=Op.subtract)
            nc.vector.tensor_tensor(d24[:, :, 0:2], dxy3, dxy3, op=Op.mult)

            # stream pairing (bf16 2x): q = 1 + e^x, two pairing levels so
            # the ACT ln pass covers NOBJ/4 elements; pinned behind btob so
            # it cannot preempt the latency-critical chain
            q = mp.tile([P, NOBJ], bf16)
            q_i = nc.vector.tensor_scalar(q[:], et[:], 1.0, None, Op.add)
            add_dep_helper(q_i.ins, btob_i.ins, False,
                           "stream pairing after the responsible-box chain")
            m1 = mp.tile([P, HALF], bf16)
            nc.vector.tensor_tensor(
                m1[:], q[:, 0:HALF], q[:, HALF:NOBJ], op=Op.mult)
            m2 = mp.tile([P, QRT], bf16)
            nc.vector.tensor_tensor(
                m2[:], m1[:, 0:QRT], m1[:, QRT:HALF], op=Op.mult)
            # class sums: cls_loss = sum(ls*v) - sum(cls*onehot_masked)
            sm = mp.tile([P, JJ], f32)
            nc.vector.tensor_reduce(sm[:], ecls3, axis=AxX, op=Op.add)
            pickv = mp.tile([P, NCLS * JJ], f32)
            pickv3 = pickv[:].rearrange("p (j c) -> p j c", c=NCLS)
            nc.vector.scalar_tensor_tensor(
                out=pickv3, in0=oh3, scalar=1.0, in1=cls3,
                op0=Op.mult, op1=Op.mult, accum_out=stats[:, 5:6])
            # noobj dedup correction
            wvb = wv.unsqueeze(2).to_broadcast([P, JJ, 2])
            corrv = mp.tile([P, 32], f32)
            corrv3 = corrv[:].rearrange("p (j k) -> p j k", k=2)
            nc.vector.scalar_tensor_tensor(
                out=corrv3, in0=scn[:].rearrange("p (j k) -> p j k", k=2),
                scalar=1.0, in1=wvb,
                op0=Op.mult, op1=Op.mult, accum_out=stats[:, 3:4])

            # ---- ACT per-GT round 2: obj pair first (btob is ready before
            #      b), then the coord lns, then the class ln
            eo = mp.tile([P, JJ], f32)
            nc.scalar.activation(out=eo[:], in_=btob[:], func=Act.Exp, scale=-1.0)
            so = mp.tile([P, JJ], f32)
            nc.scalar.activation(out=so[:], in_=eo[:], func=Act.Ln, bias=1.0)
            lnp = mp.tile([P, 32], f32)
            lnp3 = lnp[:].rearrange("p (j c) -> p j c", c=2)
            nc.scalar.activation(
                out=lnp3, in_=b3[:, :, 2:4], func=Act.Ln,
                bias=meta[:, M_EPS:M_EPS + 1])
            syp = mp.tile([P, 32], f32)
            syp_i = nc.scalar.activation(
                out=syp[:], in_=lnp[:], func=Act.Exp, scale=0.5)
            ls = mp.tile([P, JJ], f32)
            nc.scalar.activation(
                out=ls[:], in_=sm[:], func=Act.Ln,
                bias=meta[:, M_EPS:M_EPS + 1])

            # ---- ACT stream pass 2: ln of the level-2 products, accumulated;
            #      pinned after the (latency-critical) round-2 activations
            lnm = mp.tile([P, QRT], f32)
            lnm_i = nc.scalar.activation(
                out=lnm[:], in_=m2[:], func=Act.Ln,
                accum_out=stats[:, 4:5])
            add_dep_helper(lnm_i.ins, syp_i.ins, False,
                           "stream ln after per-GT round 2")

            # ---- DVE tail: coord/obj/class accumulations
            dwh = mp.tile([P, 32], f32)
            dwh3 = dwh[:].rearrange("p (j c) -> p j c", c=2)
            nc.vector.tensor_tensor(
                dwh3, syp[:].rearrange("p (j c) -> p j c", c=2), syg3,
                op=Op.subtract)
            nc.vector.tensor_tensor(d24[:, :, 2:4], dwh3, dwh3, op=Op.mult)
            gtvb = gtv.unsqueeze(2).to_broadcast([P, JJ, 4])
            coordv = mp.tile([P, 64], f32)
            coordv3 = coordv[:].rearrange("p (j m) -> p j m", m=4)
            nc.vector.scalar_tensor_tensor(
                out=coordv3, in0=d24, scalar=1.0, in1=gtvb,
                op0=Op.mult, op1=Op.mult, accum_out=stats[:, 0:1])
            objv = mp.tile([P, JJ], f32)
            nc.vector.scalar_tensor_tensor(
                out=objv[:], in0=so[:], scalar=1.0, in1=gtv,
                op0=Op.mult, op1=Op.mult, accum_out=stats[:, 1:2])
            lsv = mp.tile([P, JJ], f32)
            nc.vector.scalar_tensor_tensor(
                out=lsv[:], in0=ls[:], scalar=1.0, in1=gtv,
                op0=Op.mult, op1=Op.mult, accum_out=stats[:, 2:3])

            # ---- cross-partition reduce: ones^T @ stats
            ps = pp.tile([1, 8], f32)
            nc.tensor.matmul(
                out=ps[:], lhsT=meta[:, M_ONE:M_ONE + 1], rhs=stats[:],
                start=True, stop=True)
            outt = mp.tile([1, 8], f32)
            nc.vector.tensor_copy(out=outt[:], in_=ps[:])
            nc.sync.dma_start(out=out_d[:], in_=outt[:])

    nc.compile()
    return nc


_NC_CACHE = {}


def _get_program(for_sim: bool = False) -> bass.Bass:
    key = bool(for_sim)
    if key not in _NC_CACHE:
        _NC_CACHE[key] = build_program(for_sim)
    return _NC_CACHE[key]


def make_in_maps(predictions, gt_boxes, gt_labels, gt_valid):
    predictions = np.ascontiguousarray(np.asarray(predictions), np.float32)
    gtb = np.ascontiguousarray(np.asarray(gt_boxes), np.float32)
    gtl = np.asarray(gt_labels).astype(np.int64)
    gtv = np.asarray(gt_valid).astype(bool)
    f52 = np.float32(S)
    in_maps = []
    for c in range(NCORES):
        sl = slice(c * NIMG, (c + 1) * NIMG)
        pred = predictions[sl].reshape(ROWS, 18)
        # compact objectness stream, fp8 e4m3
        obj = np.ascontiguousarray(pred[:, 0:10:5]).reshape(P, NOBJ)
        obj = obj.astype(ml_dtypes.float8_e4m3)

        b = gtb[sl].reshape(NG, 4)
        cx, cy, w, h = b[:, 0], b[:, 1], b[:, 2], b[:, 3]
        # same float32 ops the reference does: floor(clip) of cx*S / cy*S
        gj = np.clip(np.floor(cx * f52), 0, S - 1).astype(np.float32)
        gi = np.clip(np.floor(cy * f52), 0, S - 1).astype(np.float32)
        g = np.arange(NG)
        row = ((g // N_GT) * CELLS + gi.astype(np.int64) * S
               + gj.astype(np.int64))
        # host gather of the GT cells, channel-blocked j-major
        cg = pred[row]                                   # (NG, 18)

        v = gtv[sl].reshape(NG)
        # dedup: count each GT cell once per image (first valid GT wins)
        cell_img = row.reshape(NIMG, N_GT)
        vi = v.reshape(NIMG, N_GT)
        same = cell_img[:, :, None] == cell_img[:, None, :]   # (I, j, q)
        tri = np.tril(np.ones((N_GT, N_GT), bool), -1)        # q < j
        dup = (same & vi[:, None, :] & tri[None]).any(axis=2)
        wv = (vi & ~dup).reshape(NG).astype(np.float32)

        half = np.float32(0.5)
        sjx = (gj / f52).astype(np.float32)
        sjy = (gi / f52).astype(np.float32)
        g1x, g1y = cx - w * half - sjx, cy - h * half - sjy
        g2x, g2y = cx + w * half - sjx, cy + h * half - sjy
        u1x, u1y = cx - w * half, cy - h * half
        u2x, u2y = cx + w * half, cy + h * half
        a2e = ((u2x - u1x) * (u2y - u1y) + np.float32(EPS)).astype(np.float32)
        syw = np.sqrt(w + np.float32(EPS), dtype=np.float32)
        syh = np.sqrt(h + np.float32(EPS), dtype=np.float32)

        lab = gtl[sl].reshape(NG)
        oh = ((lab[:, None] == np.arange(NCLS)[None, :])
              & (v[:, None])).astype(np.float32)

        meta = np.zeros((P, MW), np.float32)
        meta[:, C_TXY:C_TXY + 4 * JJ] = -cg[:, [1, 2, 6, 7]].reshape(P, 4 * JJ)
        meta[:, C_TWH:C_TWH + 4 * JJ] = cg[:, [3, 4, 8, 9]].reshape(P, 4 * JJ)
        meta[:, M_ONE] = 1.0
        meta[:, M_EPS] = EPS
        meta[:, M_G1:M_G1 + 2 * JJ] = np.stack(
            [g1x, g1y], 1).reshape(P, 2 * JJ)
        meta[:, M_G2:M_G2 + 2 * JJ] = np.stack(
            [g2x, g2y], 1).reshape(P, 2 * JJ)
        meta[:, M_A2E:M_A2E + JJ] = a2e.reshape(P, JJ)
        meta[:, M_SYG:M_SYG + 2 * JJ] = np.stack(
            [syw, syh], 1).reshape(P, 2 * JJ)
        meta[:, M_GXY:M_GXY + 2 * JJ] = np.stack(
            [cx - sjx, cy - sjy], 1).reshape(P, 2 * JJ)

        metab = np.zeros((P, MBW), np.float32)
        metab[:, CB_CLS:CB_CLS + NCLS * JJ] = cg[:, 10:18].reshape(P, NCLS * JJ)
        metab[:, CB_OBJ:CB_OBJ + 2 * JJ] = cg[:, [0, 5]].reshape(P, 2 * JJ)
        metab[:, B_GJS:B_GJS + 2 * JJ] = np.stack(
            [gj, gi], 1).reshape(P, 2 * JJ)
        metab[:, B_WV:B_WV + JJ] = wv.reshape(P, JJ)
        metab[:, B_GTV:B_GTV + JJ] = v.astype(np.float32).reshape(P, JJ)
        metab[:, B_OH:B_OH + NCLS * JJ] = oh.reshape(P, NCLS * JJ)

        in_maps.append({
            "obj": np.ascontiguousarray(obj),
            "mf": np.ascontiguousarray(meta),
            "mb": np.ascontiguousarray(metab.astype(ml_dtypes.bfloat16)),
        })
    return in_maps


def combine_outputs(outs):
    """outs: list of (1, 8) per-core partials -> 5-tuple of scalars."""
    t = np.stack([np.asarray(o).reshape(8) for o in outs]).astype(np.float64)
    s = t.sum(0)
    coord, obj, corr, stream = s[0], s[1], s[3], s[4]
    cls = s[2] - s[5]
    noobj = stream - corr
    total = (LAMBDA_COORD * coord + obj + LAMBDA_NOOBJ * noobj + cls) / BATCH
    return (np.float32(total), np.float32(coord / BATCH),
            np.float32(obj / BATCH), np.float32(noobj / BATCH),
            np.float32(cls / BATCH))


def kernel(predictions, gt_boxes, gt_labels, gt_valid):
    from concourse.bass_utils import run_bass_kernel_spmd

    nc = _get_program(for_sim=False)
    in_maps = make_in_maps(predictions, gt_boxes, gt_labels, gt_valid)
    try:
        res = run_bass_kernel_spmd(nc, in_maps, list(range(NCORES))).results
    except Exception:
        # transient NRT_EXEC_UNIT_UNRECOVERABLE has been observed right
        # after an earlier crashed run; one retry clears it
        res = run_bass_kernel_spmd(nc, in_maps, list(range(NCORES))).results
    return combine_outputs([r["out"] for r in res])
